# revision 1
# baseline (speedup 1.0000x reference)
"""Trainium2 Bass kernel for nn_EnhancedKeyFrameSelector.

kernel(x, params) -> (selected [2,4,64,64,10] f32, final_idx [2,10] i32,
                      div_loss f32 scalar)

Strategy: the 5 conv3d layers + per-frame scoring (>99.9% of FLOPs) run on
8 NeuronCores, sharded (batch x D-slab) = 2 x 4, fp32 end-to-end (the
selection margins are ~1e-6 so bf16/tf32 flip decisions). Each core streams
its D-slab plane-by-plane through SBUF: conv3d = 27 accumulating PE matmuls
per plane over shifted access-pattern windows of zero-padded 66x66 planes,
with taps pair/quad-packed into K=128 where possible. BN+ReLU fold into
ScalarE activations (scale/bias tables, with zero-masking of halo planes).
The tiny host tail (1-D NMS over 160 scores, top-k, fusion MLP on 30 ints,
frame gather, diversity loss) exactly mirrors the reference.
"""

import numpy as np
from contextlib import ExitStack

import concourse.bass as bass
import concourse.mybir as mybir
import concourse.tile as tile
from concourse import bacc, bass_utils

F32 = mybir.dt.float32
AF = mybir.ActivationFunctionType

# ---------------- problem geometry (hardcoded) ----------------
B, C, H, W, D_TOT = 2, 4, 64, 64, 160
NSLAB = 4
D_OUT = D_TOT // NSLAB          # 40 planes per core
N_CORES = B * NSLAB             # 8
K_SEL = 10
BN_EPS = 1e-5

G = 66                          # padded grid side
PL = G * G                      # 4356
HEAD = 67
PLW = HEAD + PL + HEAD          # 4490
INT_OFF = 2 * HEAD              # 134 = offset of interior pixel (0,0)
NCHUNK, ROWS, CHW = 8, 8, 512

# conv2/conv3 tap groups (64-ch input, pairs via upper half shifted +1)
PAIR_GROUPS = [(-67, True), (-1, True), (65, True),
               (-65, False), (1, False), (67, False)]
# attn conv2 quad shifts for stacked tile [g1, g1<<1, g1<<2, g1<<66]
QUAD_SHIFTS = [0, 1, 2, 66]
QUAD_GROUPS = [(-67, [1, 1, 1, 1]), (0, [1, 1, 0, 1]), (65, [1, 0, 1, 0])]


def _np(a):
    return np.asarray(a, dtype=np.float32)


def _delta_to_dydx(delta):
    for dy in range(3):
        for dx in range(3):
            if 66 * (dy - 1) + (dx - 1) == delta:
                return dy, dx
    raise ValueError(delta)


# ---------------- host-side weight packing ----------------

def _bn_affine(p, conv_bias):
    g, b, m, v = _np(p['g']), _np(p['b']), _np(p['m']), _np(p['v'])
    s = g / np.sqrt(v + BN_EPS)
    return s, (_np(conv_bias) - m) * s + b


def _pack_conv1(w1c, w1a):
    w1c, w1a = _np(w1c), _np(w1a)
    out = np.zeros((108, 96), np.float32)
    for dz in range(3):
        for dy in range(3):
            for dx in range(3):
                for c in range(4):
                    r = ((dz * 3 + dy) * 3 + dx) * 4 + c
                    out[r, 0:64] = w1c[:, c, dz, dy, dx]
                    out[r, 64:96] = w1a[:, c, dz, dy, dx]
    return out


def _pack_pair(w):
    w = _np(w)
    O = w.shape[0]
    out = np.zeros((3 * len(PAIR_GROUPS), 128, O), np.float32)
    for dz in range(3):
        for gi, (base, has_pair) in enumerate(PAIR_GROUPS):
            gidx = dz * len(PAIR_GROUPS) + gi
            dy, dx = _delta_to_dydx(base)
            out[gidx, 0:64, :] = w[:, :, dz, dy, dx].T
            if has_pair:
                dy2, dx2 = _delta_to_dydx(base + 1)
                out[gidx, 64:128, :] = w[:, :, dz, dy2, dx2].T
    return out


def _pack_quad(w):
    w = _np(w)
    O = w.shape[0]
    out = np.zeros((3 * len(QUAD_GROUPS), 128, O), np.float32)
    for dz in range(3):
        for gi, (base, active) in enumerate(QUAD_GROUPS):
            gidx = dz * len(QUAD_GROUPS) + gi
            for h, (sh, act) in enumerate(zip(QUAD_SHIFTS, active)):
                if act:
                    dy, dx = _delta_to_dydx(base + sh)
                    out[gidx, 32 * h:32 * (h + 1), :] = w[:, :, dz, dy, dx].T
    return out


def _make_xpad(x):
    """x [B,C,H,W,D] -> per-core [4, D_OUT+6, PLW] zero-padded plane stacks."""
    x = _np(x)
    D_IN = D_OUT + 6
    xp = np.transpose(x, (0, 1, 4, 2, 3))           # [B, C, D, H, W]
    padded = np.zeros((B, C, D_TOT + 6, PLW), np.float32)
    grid = np.zeros((B, C, D_TOT, G, G), np.float32)
    grid[:, :, :, 1:65, 1:65] = xp
    padded[:, :, 3:3 + D_TOT, HEAD:HEAD + PL] = grid.reshape(B, C, D_TOT, PL)
    return [np.ascontiguousarray(padded[b, :, s * D_OUT:s * D_OUT + D_IN, :])
            for b in range(B) for s in range(NSLAB)]


def _make_scale_bias(params, z0):
    D_L1 = D_OUT + 4
    D_L2A = D_OUT + 2
    s_c1, b_c1 = _bn_affine(params['content']['bn1'], params['content']['b1'])
    s_a1, b_a1 = _bn_affine(params['attn']['bn1'], params['attn']['b1'])
    s_c2, b_c2 = _bn_affine(params['content']['bn2'], params['content']['b2'])
    s_a2, b_a2 = _bn_affine(params['attn']['bn2'], params['attn']['b2'])
    s_a3, b_a3 = _bn_affine(params['attn']['bn3'], params['attn']['b3'])
    sc1 = np.zeros((96, D_L1), np.float32)
    bi1 = np.zeros((96, D_L1), np.float32)
    for j in range(D_L1):
        v = 1.0 if 0 <= (z0 - 2 + j) < D_TOT else 0.0
        sc1[0:64, j] = s_c1 * v
        bi1[0:64, j] = b_c1 * v
        sc1[64:96, j] = s_a1 * v
        bi1[64:96, j] = b_a1 * v
    sc2a = np.zeros((64, D_L2A), np.float32)
    bi2a = np.zeros((64, D_L2A), np.float32)
    for j in range(D_L2A):
        v = 1.0 if 0 <= (z0 - 1 + j) < D_TOT else 0.0
        sc2a[:, j] = s_a2 * v
        bi2a[:, j] = b_a2 * v
    return dict(
        sc1=sc1, bi1=bi1, sc2a=sc2a, bi2a=bi2a,
        sc2c=np.tile(s_c2[:, None], (1, D_OUT)).astype(np.float32),
        bi2c=np.tile(b_c2[:, None], (1, D_OUT)).astype(np.float32),
        sc3=np.tile(s_a3[:, None], (1, D_OUT)).astype(np.float32),
        bi3=np.tile(b_a3[:, None], (1, D_OUT)).astype(np.float32),
    )


def _make_mlp_weights(params):
    a, s = params['area'], params['scorer']
    aw1 = _np(a['w1']).T / (H * W)              # fold plane-mean
    ab1 = _np(a['b1']).reshape(2, 128).T
    aw2 = np.stack([_np(a['w2'])[:, 0:128].T, _np(a['w2'])[:, 128:256].T], axis=1)
    ab2 = _np(a['b2'])[:, None]
    aw3 = _np(a['w3']).T
    ab3 = _np(a['b3']).reshape(1, 1)
    w1s = _np(s['w1']) / 256.0                  # fold 16x16 pooling mean
    sw1 = np.zeros((128, 16, 2, 128), np.float32)
    for p in range(16):
        cols = w1s[:, np.arange(128) * 16 + p]
        sw1[:, p, 0, :] = cols[0:128, :].T
        sw1[:, p, 1, :] = cols[128:256, :].T
    sb1 = _np(s['b1']).reshape(2, 128).T
    sw2 = np.stack([_np(s['w2'])[0, 0:128], _np(s['w2'])[0, 128:256]], axis=1)
    sb2 = _np(s['b2']).reshape(1, 1)
    return dict(aw1=aw1, ab1=ab1, aw2=aw2, ab2=ab2, aw3=aw3, ab3=ab3,
                sw1=sw1, sb1=sb1, sw2=sw2, sb2=sb2)


def _prep_core_inputs(x, params):
    xpads = _make_xpad(x)
    mlp = _make_mlp_weights(params)
    w1s = _pack_conv1(params['content']['w1'], params['attn']['w1'])
    wc2 = np.ascontiguousarray(_pack_pair(params['content']['w2']).transpose(1, 0, 2))
    wa2 = np.ascontiguousarray(_pack_quad(params['attn']['w2']).transpose(1, 0, 2))
    wc3 = np.ascontiguousarray(_pack_pair(params['attn']['w3']).transpose(1, 0, 2))

    def c(a):
        return np.ascontiguousarray(a, dtype=np.float32)

    shared = dict(
        w1c=c(w1s[:, 0:64]), w1a=c(w1s[:, 64:96]),
        wc2=c(wc2), wa2=c(wa2), wc3=c(wc3),
        aw1=c(mlp['aw1']), ab1=c(mlp['ab1']), aw2=c(mlp['aw2']),
        ab2=c(mlp['ab2']), aw3=c(mlp['aw3']), ab3=c(mlp['ab3']),
        sw1=c(mlp['sw1']), sb1=c(mlp['sb1']), sw2=c(mlp['sw2']),
        sb2=c(mlp['sb2']),
    )
    in_maps = []
    ci = 0
    for b in range(B):
        for s in range(NSLAB):
            sb = _make_scale_bias(params, s * D_OUT)
            m = dict(shared)
            m['sc1c'] = c(sb['sc1'][0:64, 1:1 + D_OUT + 2])
            m['bi1c'] = c(sb['bi1'][0:64, 1:1 + D_OUT + 2])
            m['sc1a'] = c(sb['sc1'][64:96, :])
            m['bi1a'] = c(sb['bi1'][64:96, :])
            for k in ('sc2a', 'bi2a', 'sc2c', 'bi2c', 'sc3', 'bi3'):
                m[k] = c(sb[k])
            m['xpad'] = c(xpads[ci])
            in_maps.append(m)
            ci += 1
    return in_maps


# ---------------- device kernel ----------------

def build_kernel(reps=1):
    D_IN = D_OUT + 6
    D_H1 = D_OUT + 2
    D_G1 = D_OUT + 4
    D_G2 = D_OUT + 2
    NBX, NBR = 2, 3
    NG2 = 3 * len(PAIR_GROUPS)
    NGA = 3 * len(QUAD_GROUPS)

    nc = bacc.Bacc("TRN2", target_bir_lowering=False, debug=False,
                   num_devices=N_CORES)

    def din(name, shape):
        return nc.dram_tensor(name, shape, F32, kind="ExternalInput")

    xpad_d = din("xpad", [4, D_IN, PLW])
    w1c_d = din("w1c", [108, 64])
    w1a_d = din("w1a", [108, 32])
    wc2_d = din("wc2", [128, NG2, 128])
    wa2_d = din("wa2", [128, NGA, 64])
    wc3_d = din("wc3", [128, NG2, 128])
    sc1c_d = din("sc1c", [64, D_H1])
    bi1c_d = din("bi1c", [64, D_H1])
    sc1a_d = din("sc1a", [32, D_G1])
    bi1a_d = din("bi1a", [32, D_G1])
    sc2a_d = din("sc2a", [64, D_G2])
    bi2a_d = din("bi2a", [64, D_G2])
    sc2c_d = din("sc2c", [128, D_OUT])
    bi2c_d = din("bi2c", [128, D_OUT])
    sc3_d = din("sc3", [128, D_OUT])
    bi3_d = din("bi3", [128, D_OUT])
    aw1_d = din("aw1", [128, 256])
    ab1_d = din("ab1", [128, 2])
    aw2_d = din("aw2", [128, 2, 128])
    ab2_d = din("ab2", [128, 1])
    aw3_d = din("aw3", [128, 1])
    ab3_d = din("ab3", [1, 1])
    sw1_d = din("sw1", [128, 16, 2, 128])
    sb1_d = din("sb1", [128, 2])
    sw2_d = din("sw2", [128, 2])
    sb2_d = din("sb2", [1, 1])
    cs_d = nc.dram_tensor("cscore", [1, D_OUT], F32, kind="ExternalOutput")
    as_d = nc.dram_tensor("ascore", [1, D_OUT], F32, kind="ExternalOutput")

    X108 = nc.alloc_sbuf_tensor("X108", [108, NBX, PLW], F32)
    RA = nc.alloc_sbuf_tensor("RA", [128, NBR, PLW], F32)
    RB = nc.alloc_sbuf_tensor("RB", [128, NBR, PLW], F32)
    cfa = nc.alloc_sbuf_tensor("cfa", [128, D_OUT, NCHUNK], F32)
    cf = nc.alloc_sbuf_tensor("cf", [128, D_OUT], F32)
    af32 = nc.alloc_sbuf_tensor("af32", [128, 16, D_OUT], F32)
    a1 = nc.alloc_sbuf_tensor("a1", [128, 2, D_OUT], F32)
    a2 = nc.alloc_sbuf_tensor("a2", [128, D_OUT], F32)
    s1 = nc.alloc_sbuf_tensor("s1", [128, 2, D_OUT], F32)
    csb = nc.alloc_sbuf_tensor("csb", [1, D_OUT], F32)
    asb = nc.alloc_sbuf_tensor("asb", [1, D_OUT], F32)

    def ring_win(handle, slot, free_off, part_n, rows=ROWS):
        ap = handle.ap()
        return bass.AP(
            tensor=ap.tensor,
            offset=ap.offset + slot * PLW + free_off,
            ap=[[ap.ap[0][0], part_n], [66, rows], [1, 64]],
        )

    def load_x(j, slot):
        xap = xpad_d.ap()
        for dz in range(3):
            for dy in range(3):
                src = bass.AP(
                    tensor=xap.tensor,
                    offset=(j - 1 + dz) * PLW + 66 * dy,
                    ap=[[1, 3], [D_IN * PLW, 4], [1, 4356]],
                )
                p0 = dz * 36 + dy * 12
                nc.sync.dma_start(
                    out=X108.ap()[p0:p0 + 12, slot, 67:67 + 4356], in_=src)

    with tile.TileContext(nc) as tc, ExitStack() as ctx:
        singles = ctx.enter_context(tc.tile_pool(name="singles", bufs=1))
        psA = ctx.enter_context(tc.tile_pool(name="psA", bufs=2, space="PSUM"))
        psB = ctx.enter_context(tc.tile_pool(name="psB", bufs=2, space="PSUM"))
        psC = ctx.enter_context(tc.tile_pool(name="psC", bufs=2, space="PSUM"))
        psD = ctx.enter_context(tc.tile_pool(name="psD", bufs=2, space="PSUM"))
        scratch = ctx.enter_context(tc.tile_pool(name="scratch", bufs=2))
        g3pool = ctx.enter_context(tc.tile_pool(name="g3p", bufs=1))
        wpool = ctx.enter_context(tc.tile_pool(name="wpool", bufs=2))

        def load(dram, shape, tag):
            t = singles.tile(shape, F32, tag=tag)
            nc.sync.dma_start(out=t[:], in_=dram.ap())
            return t

        w1c_t = load(w1c_d, [108, 64], "w1c")
        w1a_t = load(w1a_d, [108, 32], "w1a")
        wc2_t = load(wc2_d, [128, NG2, 128], "wc2")
        wa2_t = load(wa2_d, [128, NGA, 64], "wa2")
        wc3_t = load(wc3_d, [128, NG2, 128], "wc3")
        sc1c_t = load(sc1c_d, [64, D_H1], "sc1c")
        bi1c_t = load(bi1c_d, [64, D_H1], "bi1c")
        sc1a_t = load(sc1a_d, [32, D_G1], "sc1a")
        bi1a_t = load(bi1a_d, [32, D_G1], "bi1a")
        sc2a_t = load(sc2a_d, [64, D_G2], "sc2a")
        bi2a_t = load(bi2a_d, [64, D_G2], "bi2a")
        sc2c_t = load(sc2c_d, [128, D_OUT], "sc2c")
        bi2c_t = load(bi2c_d, [128, D_OUT], "bi2c")
        sc3_t = load(sc3_d, [128, D_OUT], "sc3")
        bi3_t = load(bi3_d, [128, D_OUT], "bi3")
        aw1_t = load(aw1_d, [128, 256], "aw1")
        ab1_t = load(ab1_d, [128, 2], "ab1")
        aw2_t = load(aw2_d, [128, 2, 128], "aw2")
        ab2_t = load(ab2_d, [128, 1], "ab2")
        aw3_t = load(aw3_d, [128, 1], "aw3")
        ab3_t = load(ab3_d, [1, 1], "ab3")
        sb1_t = load(sb1_d, [128, 2], "sb1")
        sw2_t = load(sw2_d, [128, 2], "sw2")
        sb2_t = load(sb2_d, [1, 1], "sb2")

        def p1_conv1(i1):
            slot = i1 % NBX
            zs = i1 % NBR
            load_x(i1 + 2, slot)
            for n in range(NCHUNK):
                ps = psA.tile([64, CHW], F32, tag="pA")
                rhs = ring_win(X108, slot, INT_OFF + 66 * ROWS * n, 108)
                nc.tensor.matmul(ps[:], w1c_t[:], rhs, start=True, stop=True)
                psv = ps[:].rearrange("p (r c) -> p r c", r=ROWS)
                nc.scalar.activation(
                    ring_win(RA, zs, INT_OFF + 66 * ROWS * n, 64), psv, AF.Relu,
                    bias=bi1c_t[:, i1:i1 + 1], scale=sc1c_t[:, i1:i1 + 1])
            t = RA.ap()
            nc.vector.tensor_copy(t[64:128, zs, 67:4423], t[0:64, zs, 68:4424])

        def p1_conv2(i2c):
            for n in range(NCHUNK):
                ps = psB.tile([128, CHW], F32, tag="pB")
                k = 0
                for d in range(3):
                    tslot = (i2c + d) % NBR
                    for gi, (base, _pair) in enumerate(PAIR_GROUPS):
                        g = d * len(PAIR_GROUPS) + gi
                        rhs = ring_win(RA, tslot,
                                       INT_OFF + 66 * ROWS * n + base, 128)
                        nc.tensor.matmul(ps[:], wc2_t[:, g, :], rhs,
                                         start=(k == 0), stop=(k == NG2 - 1))
                        k += 1
                hs = scratch.tile([128, CHW], F32, tag="h2s")
                nc.scalar.activation(
                    hs[:], ps[:], AF.Relu,
                    bias=bi2c_t[:, i2c:i2c + 1], scale=sc2c_t[:, i2c:i2c + 1],
                    accum_out=cfa.ap()[:, i2c, n:n + 1])

        def p2_conv1(i1):
            slot = i1 % NBX
            zs = i1 % NBR
            load_x(i1 + 1, slot)
            for n in range(NCHUNK):
                ps = psA.tile([32, CHW], F32, tag="pA")
                rhs = ring_win(X108, slot, INT_OFF + 66 * ROWS * n, 108)
                nc.tensor.matmul(ps[:], w1a_t[:], rhs, start=True, stop=True)
                psv = ps[:].rearrange("p (r c) -> p r c", r=ROWS)
                nc.scalar.activation(
                    ring_win(RA, zs, INT_OFF + 66 * ROWS * n, 32), psv, AF.Relu,
                    bias=bi1a_t[:, i1:i1 + 1], scale=sc1a_t[:, i1:i1 + 1])
            q = RA.ap()
            for h, sh in enumerate(QUAD_SHIFTS):
                if h:
                    nc.vector.tensor_copy(q[32 * h:32 * (h + 1), zs, 67:4423],
                                          q[0:32, zs, 67 + sh:4423 + sh])

        def p2_conv2(i2a):
            zs = i2a % NBR
            for n in range(NCHUNK):
                ps = psB.tile([64, CHW], F32, tag="pB")
                k = 0
                for d in range(3):
                    qslot = (i2a + d) % NBR
                    for gi, (base, _act) in enumerate(QUAD_GROUPS):
                        g = d * len(QUAD_GROUPS) + gi
                        rhs = ring_win(RA, qslot,
                                       INT_OFF + 66 * ROWS * n + base, 128)
                        nc.tensor.matmul(ps[:], wa2_t[:, g, :], rhs,
                                         start=(k == 0), stop=(k == NGA - 1))
                        k += 1
                psv = ps[:].rearrange("p (r c) -> p r c", r=ROWS)
                nc.scalar.activation(
                    ring_win(RB, zs, INT_OFF + 66 * ROWS * n, 64), psv, AF.Relu,
                    bias=bi2a_t[:, i2a:i2a + 1], scale=sc2a_t[:, i2a:i2a + 1])
            t = RB.ap()
            nc.vector.tensor_copy(t[64:128, zs, 67:4423], t[0:64, zs, 68:4424])

        def p2_conv3(i3):
            g3t = g3pool.tile([128, 4096], F32, tag="g3")
            for n in range(NCHUNK):
                ps = psC.tile([128, CHW], F32, tag="pC")
                k = 0
                for d in range(3):
                    bslot = (i3 + d) % NBR
                    for gi, (base, _pair) in enumerate(PAIR_GROUPS):
                        g = d * len(PAIR_GROUPS) + gi
                        rhs = ring_win(RB, bslot,
                                       INT_OFF + 66 * ROWS * n + base, 128)
                        nc.tensor.matmul(ps[:], wc3_t[:, g, :], rhs,
                                         start=(k == 0), stop=(k == NG2 - 1))
                        k += 1
                nc.scalar.activation(
                    g3t[:, CHW * n:CHW * (n + 1)], ps[:], AF.Relu,
                    bias=bi3_t[:, i3:i3 + 1], scale=sc3_t[:, i3:i3 + 1])
            r1 = scratch.tile([128, 256], F32, tag="r1")
            nc.vector.tensor_reduce(
                r1[:], g3t[:].rearrange("p (y xg xc) -> p y xg xc", y=64, xg=4),
                axis=mybir.AxisListType.X, op=mybir.AluOpType.add)
            r1v = r1[:].rearrange("p (yg yc xg) -> p yg xg yc", yg=4, yc=16)
            afv = af32.ap()[:, :, i3].rearrange("p (yg xg) -> p yg xg", yg=4)
            nc.vector.tensor_reduce(afv, r1v, axis=mybir.AxisListType.X,
                                    op=mybir.AluOpType.add)

        for _rep in range(reps):
            # ---------- pass 1: content ----------
            nc.vector.memset(RA.ap().rearrange("p a b -> p (a b)"), 0.0)
            for step in range(D_H1):
                p1_conv1(step)
                if step >= 2:
                    p1_conv2(step - 2)
            nc.vector.tensor_reduce(cf.ap(), cfa.ap(),
                                    axis=mybir.AxisListType.X,
                                    op=mybir.AluOpType.add)
            for h in range(2):
                ps = psA.tile([128, D_OUT], F32, tag="pA")
                nc.tensor.matmul(ps[:], aw1_t[:, 128 * h:128 * (h + 1)],
                                 cf.ap(), start=True, stop=True)
                nc.scalar.activation(a1.ap()[:, h, :], ps[:], AF.Relu,
                                     bias=ab1_t[:, h:h + 1])
            ps = psB.tile([128, D_OUT], F32, tag="pB")
            for h in range(2):
                nc.tensor.matmul(ps[:], aw2_t[:, h, :], a1.ap()[:, h, :],
                                 start=(h == 0), stop=(h == 1))
            nc.scalar.activation(a2.ap(), ps[:], AF.Relu, bias=ab2_t[:, 0:1])
            ps = psC.tile([1, D_OUT], F32, tag="pC")
            nc.tensor.matmul(ps[:], aw3_t[:], a2.ap(), start=True, stop=True)
            nc.scalar.activation(csb.ap(), ps[:], AF.Identity,
                                 bias=ab3_t[:, 0:1])
            nc.sync.dma_start(out=cs_d.ap(), in_=csb.ap())

            # ---------- pass 2: attention ----------
            nc.vector.memset(RA.ap().rearrange("p a b -> p (a b)"), 0.0)
            nc.vector.memset(RB.ap().rearrange("p a b -> p (a b)"), 0.0)
            for step in range(D_G1 + 2):
                if step < D_G1:
                    p2_conv1(step)
                if 2 <= step < 2 + D_G2:
                    p2_conv2(step - 2)
                if 4 <= step < 4 + D_OUT:
                    p2_conv3(step - 4)
            for h in range(2):
                ps = psD.tile([128, D_OUT], F32, tag="pD")
                for p in range(16):
                    wch = wpool.tile([128, 128], F32, tag="sw1c")
                    nc.sync.dma_start(out=wch[:], in_=sw1_d.ap()[:, p, h, :])
                    nc.tensor.matmul(ps[:], wch[:], af32.ap()[:, p, :],
                                     start=(p == 0), stop=(p == 15))
                nc.scalar.activation(s1.ap()[:, h, :], ps[:], AF.Relu,
                                     bias=sb1_t[:, h:h + 1])
            ps = psA.tile([1, D_OUT], F32, tag="pA")
            for h in range(2):
                nc.tensor.matmul(ps[:], sw2_t[:, h:h + 1], s1.ap()[:, h, :],
                                 start=(h == 0), stop=(h == 1))
            nc.scalar.activation(asb.ap(), ps[:], AF.Sigmoid,
                                 bias=sb2_t[:, 0:1])
            nc.sync.dma_start(out=as_d.ap(), in_=asb.ap())

    nc.compile()
    return nc


# ---------------- host post-processing (mirrors reference.py) -------------

def _postprocess(x, params, content_scores, attn_scores):
    x = _np(x)
    K = K_SEL
    pos = np.arange(D_TOT)

    def nms_select(scores):
        s = scores.astype(np.float32).copy()
        idxs = []
        for _ in range(K):
            i = int(np.argmax(s))
            s[np.abs(pos - i) <= 2] = -np.inf
            idxs.append(i)
        return np.sort(np.array(idxs, np.int32))

    def topk_sorted(scores):
        order = np.argsort(-scores, kind='stable')[:K]
        return np.sort(order).astype(np.int32)

    content_idx = np.stack([nms_select(content_scores[b]) for b in range(B)])
    attn_idx = np.stack([topk_sorted(attn_scores[b]) for b in range(B)])
    uniform_idx = np.floor(np.arange(K) * (D_TOT / K)).astype(np.int32)

    f = params['fusion']
    fw1, fb1 = _np(f['w1']), _np(f['b1'])
    fw2, fb2 = _np(f['w2']), _np(f['b2'])
    fw3, fb3 = _np(f['w3']), _np(f['b3'])

    def fuse(uni, con, att):
        allidx = np.concatenate([uni, con, att]).astype(np.float32)
        Dn = np.float32(allidx.max() + 1)
        enc = (allidx / Dn).astype(np.float32)
        hh = np.maximum(enc @ fw1.T + fb1, 0).astype(np.float32)
        hh = np.maximum(hh @ fw2.T + fb2, 0).astype(np.float32)
        logits = (hh @ fw3.T + fb3).astype(np.float32)
        e = np.exp(logits - logits.max())
        w = (e / e.sum()).astype(np.float32)
        sc = np.zeros(D_TOT, np.float32)
        np.add.at(sc, uni, w)
        np.add.at(sc, con, w)
        np.add.at(sc, att, w)
        cand = np.zeros(D_TOT, bool)
        cand[uni] = True
        cand[con] = True
        cand[att] = True
        sc = np.where(cand, sc, -np.inf).astype(np.float32)
        order = np.argsort(-sc, kind='stable')[:K]
        return np.sort(order).astype(np.int32)

    final_idx = np.stack([fuse(uniform_idx, content_idx[b], attn_idx[b])
                          for b in range(B)])
    selected = np.stack([x[b][:, :, :, final_idx[b]] for b in range(B)])

    frames = np.transpose(selected, (0, 4, 1, 2, 3))
    r = frames.reshape(B, K, C, 8, H // 8, 8, W // 8)
    feat = np.concatenate([r.mean(axis=(4, 6)), r.max(axis=(4, 6))], axis=2)
    feat = feat.reshape(B, K, -1).astype(np.float32)
    nrm = np.maximum(np.linalg.norm(feat, axis=2, keepdims=True), 1e-12)
    fn = feat / nrm
    sim = np.einsum('bkf,blf->bkl', fn, fn)
    tr = np.trace(sim, axis1=1, axis2=2).sum()
    div_loss = np.float32((sim.sum() - tr) / (B * K * (K - 1)))
    return selected.astype(np.float32), final_idx.astype(np.int32), div_loss


# ---------------- entry point ----------------

_NC_CACHE = {}


def _get_nc(reps=1):
    if reps not in _NC_CACHE:
        _NC_CACHE[reps] = build_kernel(reps)
    return _NC_CACHE[reps]


def _tree_np(obj):
    if isinstance(obj, dict):
        return {k: _tree_np(v) for k, v in obj.items()}
    return np.asarray(obj)


def run_scores(x, params, reps=1, trace=False):
    """Run the device kernel; returns (content_scores, attn_scores) [B, D]."""
    nc = _get_nc(reps)
    in_maps = _prep_core_inputs(x, params)
    res = bass_utils.run_bass_kernel_spmd(
        nc, in_maps, core_ids=list(range(N_CORES)), trace=trace)
    cs = np.zeros((B, D_TOT), np.float32)
    asc = np.zeros((B, D_TOT), np.float32)
    ci = 0
    for b in range(B):
        for s in range(NSLAB):
            cs[b, s * D_OUT:(s + 1) * D_OUT] = res.results[ci]['cscore'][0]
            asc[b, s * D_OUT:(s + 1) * D_OUT] = res.results[ci]['ascore'][0]
            ci += 1
    return cs, asc, res


def kernel(x, params):
    x = np.asarray(x, dtype=np.float32)
    params = _tree_np(params)
    cs, asc, _ = run_scores(x, params)
    return _postprocess(x, params, cs, asc)


# revision 5
# speedup vs baseline: 86.6880x; 86.6880x over previous
"""Trainium2 Bass kernel for nn_EnhancedKeyFrameSelector.

kernel(x, params) -> (selected [2,4,64,64,10] f32, final_idx [2,10] i32,
                      div_loss f32 scalar)

Strategy: the 5 conv3d layers + per-frame scoring (>99.9% of FLOPs) run on
8 NeuronCores, sharded (batch x D-slab) = 2 x 4, fp32 end-to-end (the
selection margins are ~1e-6 so bf16/tf32 flip decisions). Each core streams
its D-slab plane-by-plane through SBUF: conv3d = 27 accumulating PE matmuls
per plane over shifted access-pattern windows of zero-padded 66x66 planes,
with taps pair/quad-packed into K=128 where possible. BN+ReLU fold into
ScalarE activations (scale/bias tables, with zero-masking of halo planes).
The tiny host tail (1-D NMS over 160 scores, top-k, fusion MLP on 30 ints,
frame gather, diversity loss) exactly mirrors the reference.
"""

import numpy as np
from contextlib import ExitStack

import concourse.bass as bass
import concourse.mybir as mybir
import concourse.tile as tile
from concourse import bacc, bass_utils

F32 = mybir.dt.float32
AF = mybir.ActivationFunctionType

# ---------------- problem geometry (hardcoded) ----------------
B, C, H, W, D_TOT = 2, 4, 64, 64, 160
NSLAB = 4
D_OUT = D_TOT // NSLAB          # 40 planes per core
N_CORES = B * NSLAB             # 8
K_SEL = 10
BN_EPS = 1e-5

G = 66                          # padded grid side
PL = G * G                      # 4356
HEAD = 67
PLW = HEAD + PL + HEAD          # 4490
INT_OFF = 2 * HEAD              # 134 = offset of interior pixel (0,0)
NCHUNK, ROWS, CHW = 8, 8, 512

# conv2/conv3 tap groups (64-ch input, pairs via upper half shifted +1)
PAIR_GROUPS = [(-67, True), (-1, True), (65, True),
               (-65, False), (1, False), (67, False)]
# attn conv2 quad shifts for stacked tile [g1, g1<<1, g1<<2, g1<<66]
QUAD_SHIFTS = [0, 1, 2, 66]
QUAD_GROUPS = [(-67, [1, 1, 1, 1]), (0, [1, 1, 0, 1]), (65, [1, 0, 1, 0])]


def _np(a):
    return np.asarray(a, dtype=np.float32)


def _delta_to_dydx(delta):
    for dy in range(3):
        for dx in range(3):
            if 66 * (dy - 1) + (dx - 1) == delta:
                return dy, dx
    raise ValueError(delta)


# ---------------- host-side weight packing ----------------

def _bn_affine(p, conv_bias):
    g, b, m, v = _np(p['g']), _np(p['b']), _np(p['m']), _np(p['v'])
    s = g / np.sqrt(v + BN_EPS)
    return s, (_np(conv_bias) - m) * s + b


def _pack_conv1(w1c, w1a):
    w1c, w1a = _np(w1c), _np(w1a)
    out = np.zeros((108, 96), np.float32)
    for dz in range(3):
        for dy in range(3):
            for dx in range(3):
                for c in range(4):
                    r = ((dz * 3 + dy) * 3 + dx) * 4 + c
                    out[r, 0:64] = w1c[:, c, dz, dy, dx]
                    out[r, 64:96] = w1a[:, c, dz, dy, dx]
    return out


def _pack_pair(w):
    w = _np(w)
    O = w.shape[0]
    out = np.zeros((3 * len(PAIR_GROUPS), 128, O), np.float32)
    for dz in range(3):
        for gi, (base, has_pair) in enumerate(PAIR_GROUPS):
            gidx = dz * len(PAIR_GROUPS) + gi
            dy, dx = _delta_to_dydx(base)
            out[gidx, 0:64, :] = w[:, :, dz, dy, dx].T
            if has_pair:
                dy2, dx2 = _delta_to_dydx(base + 1)
                out[gidx, 64:128, :] = w[:, :, dz, dy2, dx2].T
    return out


def _pack_quad(w):
    w = _np(w)
    O = w.shape[0]
    out = np.zeros((3 * len(QUAD_GROUPS), 128, O), np.float32)
    for dz in range(3):
        for gi, (base, active) in enumerate(QUAD_GROUPS):
            gidx = dz * len(QUAD_GROUPS) + gi
            for h, (sh, act) in enumerate(zip(QUAD_SHIFTS, active)):
                if act:
                    dy, dx = _delta_to_dydx(base + sh)
                    out[gidx, 32 * h:32 * (h + 1), :] = w[:, :, dz, dy, dx].T
    return out


def _make_xpad(x):
    """x [B,C,H,W,D] -> per-core [4, D_OUT+6, PLW] zero-padded plane stacks."""
    x = _np(x)
    D_IN = D_OUT + 6
    xp = np.transpose(x, (0, 1, 4, 2, 3))           # [B, C, D, H, W]
    padded = np.zeros((B, C, D_TOT + 6, PLW), np.float32)
    grid = np.zeros((B, C, D_TOT, G, G), np.float32)
    grid[:, :, :, 1:65, 1:65] = xp
    padded[:, :, 3:3 + D_TOT, HEAD:HEAD + PL] = grid.reshape(B, C, D_TOT, PL)
    return [np.ascontiguousarray(padded[b, :, s * D_OUT:s * D_OUT + D_IN, :])
            for b in range(B) for s in range(NSLAB)]


def _make_scale_bias(params, z0):
    D_L1 = D_OUT + 4
    D_L2A = D_OUT + 2
    s_c1, b_c1 = _bn_affine(params['content']['bn1'], params['content']['b1'])
    s_a1, b_a1 = _bn_affine(params['attn']['bn1'], params['attn']['b1'])
    s_c2, b_c2 = _bn_affine(params['content']['bn2'], params['content']['b2'])
    s_a2, b_a2 = _bn_affine(params['attn']['bn2'], params['attn']['b2'])
    s_a3, b_a3 = _bn_affine(params['attn']['bn3'], params['attn']['b3'])
    sc1 = np.zeros((96, D_L1), np.float32)
    bi1 = np.zeros((96, D_L1), np.float32)
    for j in range(D_L1):
        v = 1.0 if 0 <= (z0 - 2 + j) < D_TOT else 0.0
        sc1[0:64, j] = s_c1 * v
        bi1[0:64, j] = b_c1 * v
        sc1[64:96, j] = s_a1 * v
        bi1[64:96, j] = b_a1 * v
    sc2a = np.zeros((64, D_L2A), np.float32)
    bi2a = np.zeros((64, D_L2A), np.float32)
    for j in range(D_L2A):
        v = 1.0 if 0 <= (z0 - 1 + j) < D_TOT else 0.0
        sc2a[:, j] = s_a2 * v
        bi2a[:, j] = b_a2 * v
    return dict(
        sc1=sc1, bi1=bi1, sc2a=sc2a, bi2a=bi2a,
        sc2c=np.tile(s_c2[:, None], (1, D_OUT)).astype(np.float32),
        bi2c=np.tile(b_c2[:, None], (1, D_OUT)).astype(np.float32),
        sc3=np.tile(s_a3[:, None], (1, D_OUT)).astype(np.float32),
        bi3=np.tile(b_a3[:, None], (1, D_OUT)).astype(np.float32),
    )


def _make_mlp_weights(params):
    a, s = params['area'], params['scorer']
    aw1 = _np(a['w1']).T / (H * W)              # fold plane-mean
    ab1 = _np(a['b1']).reshape(2, 128).T
    aw2 = np.stack([_np(a['w2'])[:, 0:128].T, _np(a['w2'])[:, 128:256].T], axis=1)
    ab2 = _np(a['b2'])[:, None]
    aw3 = _np(a['w3']).T
    ab3 = _np(a['b3']).reshape(1, 1)
    w1s = _np(s['w1']) / 256.0                  # fold 16x16 pooling mean
    sw1 = np.zeros((128, 16, 2, 128), np.float32)
    for p in range(16):
        cols = w1s[:, np.arange(128) * 16 + p]
        sw1[:, p, 0, :] = cols[0:128, :].T
        sw1[:, p, 1, :] = cols[128:256, :].T
    sb1 = _np(s['b1']).reshape(2, 128).T
    sw2 = np.stack([_np(s['w2'])[0, 0:128], _np(s['w2'])[0, 128:256]], axis=1)
    sb2 = _np(s['b2']).reshape(1, 1)
    return dict(aw1=aw1, ab1=ab1, aw2=aw2, ab2=ab2, aw3=aw3, ab3=ab3,
                sw1=sw1, sb1=sb1, sw2=sw2, sb2=sb2)


def _prep_core_inputs(x, params):
    xpads = _make_xpad(x)
    mlp = _make_mlp_weights(params)
    w1s = _pack_conv1(params['content']['w1'], params['attn']['w1'])
    wc2 = np.ascontiguousarray(_pack_pair(params['content']['w2']).transpose(1, 0, 2))
    wa2 = np.ascontiguousarray(_pack_quad(params['attn']['w2']).transpose(1, 0, 2))
    wc3 = np.ascontiguousarray(_pack_pair(params['attn']['w3']).transpose(1, 0, 2))

    def c(a):
        return np.ascontiguousarray(a, dtype=np.float32)

    shared = dict(
        w1c=c(w1s[:, 0:64]), w1a=c(w1s[:, 64:96]),
        wc2=c(wc2), wa2=c(wa2), wc3=c(wc3),
        aw1=c(mlp['aw1']), ab1=c(mlp['ab1']), aw2=c(mlp['aw2']),
        ab2=c(mlp['ab2']), aw3=c(mlp['aw3']), ab3=c(mlp['ab3']),
        sw1=c(mlp['sw1']), sb1=c(mlp['sb1']), sw2=c(mlp['sw2']),
        sb2=c(mlp['sb2']),
    )
    in_maps = []
    ci = 0
    for b in range(B):
        for s in range(NSLAB):
            sb = _make_scale_bias(params, s * D_OUT)
            m = dict(shared)
            m['sc1c'] = c(sb['sc1'][0:64, 1:1 + D_OUT + 2])
            m['bi1c'] = c(sb['bi1'][0:64, 1:1 + D_OUT + 2])
            m['sc1a'] = c(sb['sc1'][64:96, :])
            m['bi1a'] = c(sb['bi1'][64:96, :])
            for k in ('sc2a', 'bi2a', 'sc2c', 'bi2c', 'sc3', 'bi3'):
                m[k] = c(sb[k])
            m['xpad'] = c(xpads[ci])
            in_maps.append(m)
            ci += 1
    return in_maps


# ---------------- device kernel ----------------

def build_kernel(reps=1):
    D_IN = D_OUT + 6
    D_H1 = D_OUT + 2
    D_G1 = D_OUT + 4
    D_G2 = D_OUT + 2
    NBX, NBR = 2, 3
    NG2 = 3 * len(PAIR_GROUPS)
    NGA = 3 * len(QUAD_GROUPS)

    nc = bacc.Bacc("TRN2", target_bir_lowering=False, debug=False,
                   num_devices=N_CORES)

    def din(name, shape):
        return nc.dram_tensor(name, shape, F32, kind="ExternalInput")

    xpad_d = din("xpad", [4, D_IN, PLW])
    w1c_d = din("w1c", [108, 64])
    w1a_d = din("w1a", [108, 32])
    wc2_d = din("wc2", [128, NG2, 128])
    wa2_d = din("wa2", [128, NGA, 64])
    wc3_d = din("wc3", [128, NG2, 128])
    sc1c_d = din("sc1c", [64, D_H1])
    bi1c_d = din("bi1c", [64, D_H1])
    sc1a_d = din("sc1a", [32, D_G1])
    bi1a_d = din("bi1a", [32, D_G1])
    sc2a_d = din("sc2a", [64, D_G2])
    bi2a_d = din("bi2a", [64, D_G2])
    sc2c_d = din("sc2c", [128, D_OUT])
    bi2c_d = din("bi2c", [128, D_OUT])
    sc3_d = din("sc3", [128, D_OUT])
    bi3_d = din("bi3", [128, D_OUT])
    aw1_d = din("aw1", [128, 256])
    ab1_d = din("ab1", [128, 2])
    aw2_d = din("aw2", [128, 2, 128])
    ab2_d = din("ab2", [128, 1])
    aw3_d = din("aw3", [128, 1])
    ab3_d = din("ab3", [1, 1])
    sw1_d = din("sw1", [128, 16, 2, 128])
    sb1_d = din("sb1", [128, 2])
    sw2_d = din("sw2", [128, 2])
    sb2_d = din("sb2", [1, 1])
    cs_d = nc.dram_tensor("cscore", [1, D_OUT], F32, kind="ExternalOutput")
    as_d = nc.dram_tensor("ascore", [1, D_OUT], F32, kind="ExternalOutput")

    X108 = nc.alloc_sbuf_tensor("X108", [108, NBX, PLW], F32)
    RA = nc.alloc_sbuf_tensor("RA", [128, NBR, PLW], F32)
    RB = nc.alloc_sbuf_tensor("RB", [128, NBR, PLW], F32)
    cfa = nc.alloc_sbuf_tensor("cfa", [128, D_OUT, NCHUNK], F32)
    cf = nc.alloc_sbuf_tensor("cf", [128, D_OUT], F32)
    af32 = nc.alloc_sbuf_tensor("af32", [128, 16, D_OUT], F32)
    a1 = nc.alloc_sbuf_tensor("a1", [128, 2, D_OUT], F32)
    a2 = nc.alloc_sbuf_tensor("a2", [128, D_OUT], F32)
    s1 = nc.alloc_sbuf_tensor("s1", [128, 2, D_OUT], F32)
    csb = nc.alloc_sbuf_tensor("csb", [1, D_OUT], F32)
    asb = nc.alloc_sbuf_tensor("asb", [1, D_OUT], F32)

    def ring_win(handle, slot, free_off, part_n, rows=ROWS):
        ap = handle.ap()
        return bass.AP(
            tensor=ap.tensor,
            offset=ap.offset + slot * PLW + free_off,
            ap=[[ap.ap[0][0], part_n], [66, rows], [1, 64]],
        )

    def load_x(j, slot):
        xap = xpad_d.ap()
        for dz in range(3):
            for dy in range(3):
                src = bass.AP(
                    tensor=xap.tensor,
                    offset=(j - 1 + dz) * PLW + 66 * dy,
                    ap=[[1, 3], [D_IN * PLW, 4], [1, 4356]],
                )
                p0 = dz * 36 + dy * 12
                nc.sync.dma_start(
                    out=X108.ap()[p0:p0 + 12, slot, 67:67 + 4356], in_=src)

    with tile.TileContext(nc) as tc, ExitStack() as ctx:
        singles = ctx.enter_context(tc.tile_pool(name="singles", bufs=1))
        psA = ctx.enter_context(tc.tile_pool(name="psA", bufs=2, space="PSUM"))
        psB = ctx.enter_context(tc.tile_pool(name="psB", bufs=2, space="PSUM"))
        psC = ctx.enter_context(tc.tile_pool(name="psC", bufs=2, space="PSUM"))
        psD = ctx.enter_context(tc.tile_pool(name="psD", bufs=2, space="PSUM"))
        scratch = ctx.enter_context(tc.tile_pool(name="scratch", bufs=2))
        g3pool = ctx.enter_context(tc.tile_pool(name="g3p", bufs=1))
        wpool = ctx.enter_context(tc.tile_pool(name="wpool", bufs=2))

        def load(dram, shape, tag):
            t = singles.tile(shape, F32, tag=tag)
            nc.sync.dma_start(out=t[:], in_=dram.ap())
            return t

        w1c_t = load(w1c_d, [108, 64], "w1c")
        w1a_t = load(w1a_d, [108, 32], "w1a")
        wc2_t = load(wc2_d, [128, NG2, 128], "wc2")
        wa2_t = load(wa2_d, [128, NGA, 64], "wa2")
        wc3_t = load(wc3_d, [128, NG2, 128], "wc3")
        sc1c_t = load(sc1c_d, [64, D_H1], "sc1c")
        bi1c_t = load(bi1c_d, [64, D_H1], "bi1c")
        sc1a_t = load(sc1a_d, [32, D_G1], "sc1a")
        bi1a_t = load(bi1a_d, [32, D_G1], "bi1a")
        sc2a_t = load(sc2a_d, [64, D_G2], "sc2a")
        bi2a_t = load(bi2a_d, [64, D_G2], "bi2a")
        sc2c_t = load(sc2c_d, [128, D_OUT], "sc2c")
        bi2c_t = load(bi2c_d, [128, D_OUT], "bi2c")
        sc3_t = load(sc3_d, [128, D_OUT], "sc3")
        bi3_t = load(bi3_d, [128, D_OUT], "bi3")
        aw1_t = load(aw1_d, [128, 256], "aw1")
        ab1_t = load(ab1_d, [128, 2], "ab1")
        aw2_t = load(aw2_d, [128, 2, 128], "aw2")
        ab2_t = load(ab2_d, [128, 1], "ab2")
        aw3_t = load(aw3_d, [128, 1], "aw3")
        ab3_t = load(ab3_d, [1, 1], "ab3")
        sb1_t = load(sb1_d, [128, 2], "sb1")
        sw2_t = load(sw2_d, [128, 2], "sw2")
        sb2_t = load(sb2_d, [1, 1], "sb2")

        def p1_conv1(i1):
            slot = i1 % NBX
            zs = i1 % NBR
            load_x(i1 + 2, slot)
            for n in range(NCHUNK):
                ps = psA.tile([64, CHW], F32, tag="pA")
                rhs = ring_win(X108, slot, INT_OFF + 66 * ROWS * n, 108)
                nc.tensor.matmul(ps[:], w1c_t[:], rhs, start=True, stop=True)
                psv = ps[:].rearrange("p (r c) -> p r c", r=ROWS)
                nc.scalar.activation(
                    ring_win(RA, zs, INT_OFF + 66 * ROWS * n, 64), psv, AF.Relu,
                    bias=bi1c_t[:, i1:i1 + 1], scale=sc1c_t[:, i1:i1 + 1])
            t = RA.ap()
            nc.vector.tensor_copy(t[64:128, zs, 67:4423], t[0:64, zs, 68:4424])

        def p1_conv2(i2c):
            for n in range(NCHUNK):
                ps = psB.tile([128, CHW], F32, tag="pB")
                k = 0
                for d in range(3):
                    tslot = (i2c + d) % NBR
                    for gi, (base, _pair) in enumerate(PAIR_GROUPS):
                        g = d * len(PAIR_GROUPS) + gi
                        rhs = ring_win(RA, tslot,
                                       INT_OFF + 66 * ROWS * n + base, 128)
                        nc.tensor.matmul(ps[:], wc2_t[:, g, :], rhs,
                                         start=(k == 0), stop=(k == NG2 - 1))
                        k += 1
                hs = scratch.tile([128, CHW], F32, tag="h2s")
                nc.scalar.activation(
                    hs[:], ps[:], AF.Relu,
                    bias=bi2c_t[:, i2c:i2c + 1], scale=sc2c_t[:, i2c:i2c + 1],
                    accum_out=cfa.ap()[:, i2c, n:n + 1])

        def p2_conv1(i1):
            slot = i1 % NBX
            zs = i1 % NBR
            load_x(i1 + 1, slot)
            for n in range(NCHUNK):
                ps = psA.tile([32, CHW], F32, tag="pA")
                rhs = ring_win(X108, slot, INT_OFF + 66 * ROWS * n, 108)
                nc.tensor.matmul(ps[:], w1a_t[:], rhs, start=True, stop=True)
                psv = ps[:].rearrange("p (r c) -> p r c", r=ROWS)
                nc.scalar.activation(
                    ring_win(RA, zs, INT_OFF + 66 * ROWS * n, 32), psv, AF.Relu,
                    bias=bi1a_t[:, i1:i1 + 1], scale=sc1a_t[:, i1:i1 + 1])
            q = RA.ap()
            for h, sh in enumerate(QUAD_SHIFTS):
                if h:
                    nc.vector.tensor_copy(q[32 * h:32 * (h + 1), zs, 67:4423],
                                          q[0:32, zs, 67 + sh:4423 + sh])

        def p2_conv2(i2a):
            zs = i2a % NBR
            for n in range(NCHUNK):
                ps = psB.tile([64, CHW], F32, tag="pB")
                k = 0
                for d in range(3):
                    qslot = (i2a + d) % NBR
                    for gi, (base, _act) in enumerate(QUAD_GROUPS):
                        g = d * len(QUAD_GROUPS) + gi
                        rhs = ring_win(RA, qslot,
                                       INT_OFF + 66 * ROWS * n + base, 128)
                        nc.tensor.matmul(ps[:], wa2_t[:, g, :], rhs,
                                         start=(k == 0), stop=(k == NGA - 1))
                        k += 1
                psv = ps[:].rearrange("p (r c) -> p r c", r=ROWS)
                nc.scalar.activation(
                    ring_win(RB, zs, INT_OFF + 66 * ROWS * n, 64), psv, AF.Relu,
                    bias=bi2a_t[:, i2a:i2a + 1], scale=sc2a_t[:, i2a:i2a + 1])
            t = RB.ap()
            nc.vector.tensor_copy(t[64:128, zs, 67:4423], t[0:64, zs, 68:4424])

        def p2_conv3(i3):
            g3t = g3pool.tile([128, 4096], F32, tag="g3")
            for n in range(NCHUNK):
                ps = psC.tile([128, CHW], F32, tag="pC")
                k = 0
                for d in range(3):
                    bslot = (i3 + d) % NBR
                    for gi, (base, _pair) in enumerate(PAIR_GROUPS):
                        g = d * len(PAIR_GROUPS) + gi
                        rhs = ring_win(RB, bslot,
                                       INT_OFF + 66 * ROWS * n + base, 128)
                        nc.tensor.matmul(ps[:], wc3_t[:, g, :], rhs,
                                         start=(k == 0), stop=(k == NG2 - 1))
                        k += 1
                nc.scalar.activation(
                    g3t[:, CHW * n:CHW * (n + 1)], ps[:], AF.Relu,
                    bias=bi3_t[:, i3:i3 + 1], scale=sc3_t[:, i3:i3 + 1])
            r1 = scratch.tile([128, 256], F32, tag="r1")
            nc.vector.tensor_reduce(
                r1[:], g3t[:].rearrange("p (y xg xc) -> p y xg xc", y=64, xg=4),
                axis=mybir.AxisListType.X, op=mybir.AluOpType.add)
            r1v = r1[:].rearrange("p (yg yc xg) -> p yg xg yc", yg=4, yc=16)
            afv = af32.ap()[:, :, i3].rearrange("p (yg xg) -> p yg xg", yg=4)
            nc.vector.tensor_reduce(afv, r1v, axis=mybir.AxisListType.X,
                                    op=mybir.AluOpType.add)

        for _rep in range(reps):
            # ---------- pass 1: content ----------
            nc.vector.memset(RA.ap().rearrange("p a b -> p (a b)"), 0.0)
            for step in range(D_H1):
                p1_conv1(step)
                if step >= 2:
                    p1_conv2(step - 2)
            nc.vector.tensor_reduce(cf.ap(), cfa.ap(),
                                    axis=mybir.AxisListType.X,
                                    op=mybir.AluOpType.add)
            for h in range(2):
                ps = psA.tile([128, D_OUT], F32, tag="pA")
                nc.tensor.matmul(ps[:], aw1_t[:, 128 * h:128 * (h + 1)],
                                 cf.ap(), start=True, stop=True)
                nc.scalar.activation(a1.ap()[:, h, :], ps[:], AF.Relu,
                                     bias=ab1_t[:, h:h + 1])
            ps = psB.tile([128, D_OUT], F32, tag="pB")
            for h in range(2):
                nc.tensor.matmul(ps[:], aw2_t[:, h, :], a1.ap()[:, h, :],
                                 start=(h == 0), stop=(h == 1))
            nc.scalar.activation(a2.ap(), ps[:], AF.Relu, bias=ab2_t[:, 0:1])
            ps = psC.tile([1, D_OUT], F32, tag="pC")
            nc.tensor.matmul(ps[:], aw3_t[:], a2.ap(), start=True, stop=True)
            nc.scalar.activation(csb.ap(), ps[:], AF.Identity,
                                 bias=ab3_t[:, 0:1])
            nc.sync.dma_start(out=cs_d.ap(), in_=csb.ap())

            # ---------- pass 2: attention ----------
            nc.vector.memset(RA.ap().rearrange("p a b -> p (a b)"), 0.0)
            nc.vector.memset(RB.ap().rearrange("p a b -> p (a b)"), 0.0)
            for step in range(D_G1 + 2):
                if step < D_G1:
                    p2_conv1(step)
                if 2 <= step < 2 + D_G2:
                    p2_conv2(step - 2)
                if 4 <= step < 4 + D_OUT:
                    p2_conv3(step - 4)
            for h in range(2):
                ps = psD.tile([128, D_OUT], F32, tag="pD")
                for p in range(16):
                    wch = wpool.tile([128, 128], F32, tag="sw1c")
                    nc.sync.dma_start(out=wch[:], in_=sw1_d.ap()[:, p, h, :])
                    nc.tensor.matmul(ps[:], wch[:], af32.ap()[:, p, :],
                                     start=(p == 0), stop=(p == 15))
                nc.scalar.activation(s1.ap()[:, h, :], ps[:], AF.Relu,
                                     bias=sb1_t[:, h:h + 1])
            ps = psA.tile([1, D_OUT], F32, tag="pA")
            for h in range(2):
                nc.tensor.matmul(ps[:], sw2_t[:, h:h + 1], s1.ap()[:, h, :],
                                 start=(h == 0), stop=(h == 1))
            nc.scalar.activation(asb.ap(), ps[:], AF.Sigmoid,
                                 bias=sb2_t[:, 0:1])
            nc.sync.dma_start(out=as_d.ap(), in_=asb.ap())

    nc.compile()
    return nc


# ---------------- host post-processing (mirrors reference.py) -------------

def _postprocess(x, params, content_scores, attn_scores):
    x = _np(x)
    K = K_SEL
    pos = np.arange(D_TOT)

    def nms_select(scores):
        s = scores.astype(np.float32).copy()
        idxs = []
        for _ in range(K):
            i = int(np.argmax(s))
            s[np.abs(pos - i) <= 2] = -np.inf
            idxs.append(i)
        return np.sort(np.array(idxs, np.int32))

    def topk_sorted(scores):
        order = np.argsort(-scores, kind='stable')[:K]
        return np.sort(order).astype(np.int32)

    content_idx = np.stack([nms_select(content_scores[b]) for b in range(B)])
    attn_idx = np.stack([topk_sorted(attn_scores[b]) for b in range(B)])
    uniform_idx = np.floor(np.arange(K) * (D_TOT / K)).astype(np.int32)

    f = params['fusion']
    fw1, fb1 = _np(f['w1']), _np(f['b1'])
    fw2, fb2 = _np(f['w2']), _np(f['b2'])
    fw3, fb3 = _np(f['w3']), _np(f['b3'])

    def fuse(uni, con, att):
        allidx = np.concatenate([uni, con, att]).astype(np.float32)
        Dn = np.float32(allidx.max() + 1)
        enc = (allidx / Dn).astype(np.float32)
        hh = np.maximum(enc @ fw1.T + fb1, 0).astype(np.float32)
        hh = np.maximum(hh @ fw2.T + fb2, 0).astype(np.float32)
        logits = (hh @ fw3.T + fb3).astype(np.float32)
        e = np.exp(logits - logits.max())
        w = (e / e.sum()).astype(np.float32)
        sc = np.zeros(D_TOT, np.float32)
        np.add.at(sc, uni, w)
        np.add.at(sc, con, w)
        np.add.at(sc, att, w)
        cand = np.zeros(D_TOT, bool)
        cand[uni] = True
        cand[con] = True
        cand[att] = True
        sc = np.where(cand, sc, -np.inf).astype(np.float32)
        order = np.argsort(-sc, kind='stable')[:K]
        return np.sort(order).astype(np.int32)

    final_idx = np.stack([fuse(uniform_idx, content_idx[b], attn_idx[b])
                          for b in range(B)])
    selected = np.stack([x[b][:, :, :, final_idx[b]] for b in range(B)])

    frames = np.transpose(selected, (0, 4, 1, 2, 3))
    r = frames.reshape(B, K, C, 8, H // 8, 8, W // 8)
    feat = np.concatenate([r.mean(axis=(4, 6)), r.max(axis=(4, 6))], axis=2)
    feat = feat.reshape(B, K, -1).astype(np.float32)
    nrm = np.maximum(np.linalg.norm(feat, axis=2, keepdims=True), 1e-12)
    fn = feat / nrm
    sim = np.einsum('bkf,blf->bkl', fn, fn)
    tr = np.trace(sim, axis1=1, axis2=2).sum()
    div_loss = np.float32((sim.sum() - tr) / (B * K * (K - 1)))
    return selected.astype(np.float32), final_idx.astype(np.int32), div_loss


# ---------------- cached PJRT runner ----------------

class _Runner:
    """Compile the NEFF-backed jitted executable once; reuse across calls
    (run_bass_kernel_spmd re-traces and re-serializes the module per call,
    which dominates wall time for large modules)."""

    def __init__(self, nc):
        import jax
        from jax.sharding import Mesh, PartitionSpec
        from jax.experimental.shard_map import shard_map
        from concourse import bass2jax
        import concourse.mybir as mb

        bass2jax.install_neuronx_cc_hook()
        self.nc = nc
        partition_name = (nc.partition_id_tensor.name
                          if nc.partition_id_tensor else None)
        in_names, out_names, out_avals, zero_outs = [], [], [], []
        for alloc in nc.m.functions[0].allocations:
            if not isinstance(alloc, mb.MemoryLocationSet):
                continue
            name = alloc.memorylocations[0].name
            if alloc.kind == "ExternalInput":
                if name != partition_name:
                    in_names.append(name)
            elif alloc.kind == "ExternalOutput":
                shape = tuple(alloc.tensor_shape)
                dtype = mb.dt.np(alloc.dtype)
                out_names.append(name)
                out_avals.append(jax.core.ShapedArray(shape, dtype))
                zero_outs.append(np.zeros(shape, dtype))
        self.in_names = in_names
        self.out_names = out_names
        n_params = len(in_names)
        n_outs = len(out_avals)
        all_names = in_names + out_names
        if partition_name is not None:
            all_names = all_names + [partition_name]

        def _body(*args):
            operands = list(args)
            if partition_name is not None:
                operands.append(bass2jax.partition_id_tensor())
            outs = bass2jax._bass_exec_p.bind(
                *operands,
                out_avals=tuple(out_avals),
                in_names=tuple(all_names),
                out_names=tuple(out_names),
                lowering_input_output_aliases=(),
                sim_require_finite=True,
                sim_require_nnan=True,
                nc=nc,
            )
            return tuple(outs)

        devices = jax.devices()[:N_CORES]
        mesh = Mesh(np.asarray(devices), ("core",))
        donate = tuple(range(n_params, n_params + n_outs))
        self._fn = jax.jit(
            shard_map(_body, mesh=mesh,
                      in_specs=(PartitionSpec("core"),) * (n_params + n_outs),
                      out_specs=(PartitionSpec("core"),) * n_outs,
                      check_rep=False),
            donate_argnums=donate, keep_unused=True)
        self._zero_outs = zero_outs
        self._concat_in = None

    def set_inputs(self, in_maps):
        self._concat_in = [
            np.concatenate([np.asarray(m[name]) for m in in_maps], axis=0)
            for name in self.in_names
        ]

    def run(self):
        import jax
        zeros = [np.zeros((N_CORES * z.shape[0], *z.shape[1:]), z.dtype)
                 for z in self._zero_outs]
        out = self._fn(*self._concat_in, *zeros)
        out = [np.asarray(o) for o in out]
        return [
            {name: out[i].reshape(N_CORES, *self._zero_outs[i].shape)[c]
             for i, name in enumerate(self.out_names)}
            for c in range(N_CORES)
        ]


_NC_CACHE = {}


def _get_nc(reps=1):
    if reps not in _NC_CACHE:
        nc = build_kernel(reps)
        _NC_CACHE[reps] = _Runner(nc)
    return _NC_CACHE[reps]


def _tree_np(obj):
    if isinstance(obj, dict):
        return {k: _tree_np(v) for k, v in obj.items()}
    return np.asarray(obj)


def run_scores(x, params, reps=1, prepped=None):
    """Run the device kernel; returns (content_scores, attn_scores) [B, D]."""
    runner = _get_nc(reps)
    if prepped is None:
        prepped = _prep_core_inputs(x, params)
    runner.set_inputs(prepped)
    results = runner.run()
    cs = np.zeros((B, D_TOT), np.float32)
    asc = np.zeros((B, D_TOT), np.float32)
    ci = 0
    for b in range(B):
        for s in range(NSLAB):
            cs[b, s * D_OUT:(s + 1) * D_OUT] = results[ci]['cscore'][0]
            asc[b, s * D_OUT:(s + 1) * D_OUT] = results[ci]['ascore'][0]
            ci += 1
    return cs, asc, results


def time_device(x, params, n=3):
    """(t_reps3 - t_reps1)/2 with cached jit: isolates device exec time."""
    import time as _time
    prepped = _prep_core_inputs(x, params)
    outs = {}
    for reps in (1, 3):
        runner = _get_nc(reps)
        runner.set_inputs(prepped)
        runner.run()  # warm
        ts = []
        for _ in range(n):
            t0 = _time.time()
            runner.run()
            ts.append(_time.time() - t0)
        outs[reps] = min(ts)
    hw_s = (outs[3] - outs[1]) / 2.0
    return hw_s, outs[1], outs[3]


def kernel(x, params):
    x = np.asarray(x, dtype=np.float32)
    params = _tree_np(params)
    cs, asc, _ = run_scores(x, params)
    return _postprocess(x, params, cs, asc)
